# revision 25
# baseline (speedup 1.0000x reference)
# Fused dynamic-conv (CondInst-style) + dice loss kernel for 8x TRN2 NeuronCores.
#
# Reference computation (per batch image b, object o):
#   weight[b,o,:] = conv_weight[b, :, ind[b,o]]           (gather, 593 params)
#   feat = concat(seg_feat[b], x_rel(o), y_rel(o))        ([18, 128*128])
#   h1 = relu(w1 @ feat + b1); h2 = relu(w2 @ h1 + b2)    (16-ch dynamic 1x1 convs)
#   out = sigmoid(w3 . h2 + b3)                           ([128*128])
#   dice over masked objects -> scalar loss
#
# Strategy (v3 — evacuation-balanced pipeline):
#  * Host gathers the 593 dynamic params per object and packs active (mask=1)
#    objects into groups of 8; x_rel/y_rel fold into an effective bias b1_eff.
#    One shared [18, HW] feature map per image serves all its objects.
#  * Work unit = (group, half-image) = 8192 px; 16 units over 8 cores (NG=2).
#  * Matmuls run as 4 concurrent diagonal/column 32-wide PE tiles (full column
#    rate).  PSUM = 4 rotating [128, 1024] slots (2 banks each): fill is 8 MMs
#    (~0.5us), drain is ONE FD=1024 evacuation op (~1.1us), so two slots are
#    always draining on the two evac engines concurrently.
#  * The evacuations (relu+bias) are the roofline: ~45K FD-cycles/core split
#    across ACT and DVE; assignments alternate strictly between the engines
#    (parity flipped per unit/phase) to keep both ~balanced (~25us each).
#  * gemm3 is column-tiled and quadrant-packed per half-unit into [128, 1024]
#    PSUM slots; sigmoid evacuates fp16 pred; dice partials via accum_out:
#    sum pred*tgt (DVE scalar_tensor_tensor) and sum pred^2 (Square on ACT /
#    STT on DVE, alternating per unit).  Host does the final tiny reduction
#    plus sum(tgt^2), which is network-independent.
import numpy as np
from contextlib import ExitStack

import concourse.bass as bass
import concourse.tile as tile
from concourse import mybir, bacc
from concourse.bass_utils import run_bass_kernel_spmd

C = 16
WT = 593
B, O, H, W = 4, 32, 128, 128
HW = H * W
N_CORES = 8
GRP = 8            # objects per block-diagonal group
HALF = 8192        # pixels per work unit

F32 = mybir.dt.float32
F16 = mybir.dt.float16
ACTF = mybir.ActivationFunctionType
ALU = mybir.AluOpType

# wpack free-dim layout (per unit, [128, 192] float16):
#   0:128   lhsT1 [18, 128]: full-array gemm1, col 16o+c = w1[o][c, :].
#           Duplicated at partition rows 32:50 — gemm1 matmuls alternate the
#           two copies (different PE row groups) so each LDWEIGHTS overlaps
#           the other copy's in-flight matmul instead of serializing.
#   128:160 lhsT2 band blocks: rows 32r hold diag(w2[2r]^T, w2[2r+1]^T)
#           (gemm2 runs as 4 concurrent diagonal 32x32 PE tiles)
#   160:192 lhsT3 (block-diag w3; cols 160+8:192 zero)
# bias layout (per unit, [128, 3] float32): 0 = b1_eff, 1 = b2, 2 = b3/-50
# feat layout: [NG, 2, 18, 4096] — band p holds the 512-px moving tiles with
# t % 2 == p (concatenated), matching the gemm1 band alternation.
WCOLS = 192


def host_pack(seg_feat, conv_weight, mask, ind, target):
    cw = conv_weight.reshape(B, WT, HW)
    weight = np.take_along_axis(cw, ind[:, None, :].astype(np.int64), axis=2)
    weight = np.ascontiguousarray(weight.transpose(0, 2, 1))  # [B, O, WT]
    s0 = (C + 2) * C
    w1 = weight[..., :s0].reshape(B, O, C, C + 2)
    b1 = weight[..., s0:s0 + C]
    w2 = weight[..., s0 + C:s0 + C + C * C].reshape(B, O, C, C)
    b2 = weight[..., s0 + C + C * C:s0 + 2 * C + C * C]
    w3 = weight[..., s0 + 2 * C + C * C:s0 + 3 * C + C * C]
    b3 = weight[..., -1]
    xo = (ind % W).astype(np.float32)
    yo = (ind // W).astype(np.float32)

    units = []  # (b, objs[8 padded with -1], half)
    for b in range(B):
        objs = [o for o in range(O) if mask[b, o] == 1]
        for g0 in range(0, len(objs), GRP):
            grp = objs[g0:g0 + GRP]
            grp = grp + [-1] * (GRP - len(grp))
            for half in range(2):
                units.append((b, grp, half))
    per_core = [[] for _ in range(N_CORES)]
    for i, u in enumerate(units):
        per_core[i % N_CORES].append(u)
    NG = max(1, max(len(u) for u in per_core))
    for ci in range(N_CORES):
        while len(per_core[ci]) < NG:
            per_core[ci].append((0, [-1] * GRP, 0))

    px = np.arange(HW, dtype=np.float32)
    xg = (px % W) / 128.0
    yg = np.floor(px / W) / 128.0
    tgt_flat = target.reshape(B, O, HW)

    in_maps = []
    for ci in range(N_CORES):
        feat_pack = np.zeros((NG, 2, 18, HALF // 2), np.float16)
        wpack = np.zeros((NG, 128, WCOLS), np.float16)
        bias_pack = np.zeros((NG, 128, 3), np.float32)
        # tgt dense in the packed sigmoid layout: row (q, o) maps to
        # partition 32q+o; pred[32q+o, 512k+j] holds pixel 2048k + 512q + j
        # of object o  (t = 4k+q moving tiles)
        tgt_pack = np.zeros((NG, 4, GRP, 2048), np.float16)
        for u, (b, grp, half) in enumerate(per_core[ci]):
            sl = slice(half * HALF, (half + 1) * HALF)
            fu = np.empty((18, HALF), np.float16)
            fu[:16] = seg_feat[b].reshape(C, HW)[:, sl]
            fu[16] = xg[sl]
            fu[17] = yg[sl]
            # band p = 512-px tiles t with t % 2 == p, concatenated
            ft4 = fu.reshape(18, 16, 512)
            feat_pack[u, 0] = ft4[:, 0::2].reshape(18, HALF // 2)
            feat_pack[u, 1] = ft4[:, 1::2].reshape(18, HALF // 2)
            bias_pack[u, :, 2] = -50.0  # filler-row sigmoid bias
            for oo, o in enumerate(grp):
                if o < 0:
                    continue
                r, p = oo // 2, oo % 2
                w1T = w1[b, o].T.astype(np.float16)
                wpack[u, 0:18, 16 * oo:16 * oo + 16] = w1T
                wpack[u, 32:50, 16 * oo:16 * oo + 16] = w1T
                b1e = (b1[b, o] - w1[b, o, :, 16] * (xo[b, o] / 128.0)
                       - w1[b, o, :, 17] * (yo[b, o] / 128.0))
                bias_pack[u, 16 * oo:16 * oo + 16, 0] = b1e
                wpack[u, 32 * r + 16 * p:32 * r + 16 * p + 16,
                      128 + 16 * p:128 + 16 * p + 16] = \
                    w2[b, o].T.astype(np.float16)
                bias_pack[u, 16 * oo:16 * oo + 16, 1] = b2[b, o]
                wpack[u, 16 * oo:16 * oo + 16, 160 + oo] = \
                    w3[b, o].astype(np.float16)
                for q in range(4):
                    bias_pack[u, 32 * q + oo, 2] = b3[b, o]
                for t in range(16):
                    q, k = t % 4, t // 4
                    g0 = half * HALF + t * 512
                    tgt_pack[u, q, oo, 512 * k:512 * k + 512] = \
                        tgt_flat[b, o, g0:g0 + 512].astype(np.float16)
        in_maps.append({"feat": feat_pack, "wpack": wpack,
                        "bias": bias_pack, "tgt": tgt_pack})
    return in_maps, per_core, NG


_PROGRAM_CACHE = {}


def build_program(NG):
    if NG in _PROGRAM_CACHE:
        return _PROGRAM_CACHE[NG]
    nc = bacc.Bacc("TRN2", target_bir_lowering=False, debug=False,
                   enable_asserts=False, num_devices=N_CORES)
    feat_t = nc.dram_tensor("feat", (NG, 2, 18, HALF // 2), F16,
                            kind="ExternalInput")
    wpack_t = nc.dram_tensor("wpack", (NG, 128, WCOLS), F16, kind="ExternalInput")
    bias_t = nc.dram_tensor("bias", (NG, 128, 3), F32, kind="ExternalInput")
    tgt_t = nc.dram_tensor("tgt", (NG, 4, GRP, 2048), F16,
                           kind="ExternalInput")
    acc_t = nc.dram_tensor("acc", (128, 4 * NG), F32, kind="ExternalOutput")

    with tile.TileContext(nc) as tc, ExitStack() as ctx:
        wpool = ctx.enter_context(tc.tile_pool(name="wpool", bufs=2))
        fpool = ctx.enter_context(tc.tile_pool(name="fpool", bufs=2))
        h1pool = ctx.enter_context(tc.tile_pool(name="h1pool", bufs=2))
        h2pool = ctx.enter_context(tc.tile_pool(name="h2pool", bufs=2))
        tpool = ctx.enter_context(tc.tile_pool(name="tpool", bufs=2))
        ppool = ctx.enter_context(tc.tile_pool(name="ppool", bufs=2))
        spool = ctx.enter_context(tc.tile_pool(name="spool", bufs=4))
        apool = ctx.enter_context(tc.tile_pool(name="apool", bufs=1))
        ps = ctx.enter_context(tc.tile_pool(name="ps", bufs=4, space="PSUM"))

        # one accumulator tile: cols [0, 2NG) = inter, [2NG, 4NG) = predsq
        acc = apool.tile([128, 4 * NG], F32)

        tg_tiles = [tpool.tile([128, 2048], F16, tag="t", name=f"tg{i}")
                    for i in range(2)]
        for tgt_tile in tg_tiles:
            nc.vector.memset(tgt_tile, 0.0)

        # Warm-up during the initial DMA wait: load the sigmoid table set
        # (covers relu/sigmoid/square, so no mid-kernel table switch) and run
        # dummy matmuls back-to-back so the PE HAM clock-gate opens (needs
        # ~3.4us of sustained PE activity) before the real matmuls arrive.
        scr = apool.tile([128, 512], F16)
        nc.vector.memset(scr, 0.125)
        scr1 = apool.tile([128, 1], F32)
        nc.scalar.activation(scr1, scr[:, 0:1], ACTF.Sigmoid, bias=0.0, scale=1.0)
        pw = ps.tile([128, 1024], F32, tag="ps")
        for i in range(8):
            nc.tensor.matmul(pw[:, 512 * (i % 2):512 * (i % 2) + 512],
                             scr[:, 0:128], scr, start=True, stop=True)

        for u in range(NG):
            wt = wpool.tile([128, WCOLS], F16, tag="w")
            bt = wpool.tile([128, 3], F32, tag="b")
            nc.gpsimd.dma_start(out=bt, in_=bias_t.ap()[u])
            # gemm1 weights are only rows 0:50 of cols 0:128 — ship just
            # those (12.8KB) so the first matmul isn't gated on a big DMA.
            # (NOT on the scalar queue: ACT-queue DMAs force a spurious ACT
            # table load.)
            nc.sync.dma_start(out=wt[0:50, 0:128],
                              in_=wpack_t.ap()[u][0:50, 0:128])
            ft = fpool.tile([50, HALF // 2], F16, tag="f")
            # fine-grained feat DMAs (per band per chunk), first pixel-chunk
            # first; the very first chunk is split in half again.  Issues
            # alternate sync/gpsimd to parallelize issue + transfer.
            for p in range(2):
                seq = nc.sync if p == 0 else nc.gpsimd
                seq.dma_start(out=ft[32 * p:32 * p + 18, 0:512],
                              in_=feat_t.ap()[u][p][:, 0:512])
                seq.dma_start(out=ft[32 * p:32 * p + 18, 512:1024],
                              in_=feat_t.ap()[u][p][:, 512:1024])
            for j in range(1, 4):
                fsl = slice(1024 * j, 1024 * j + 1024)
                for p in range(2):
                    seq = nc.sync if p == 0 else nc.gpsimd
                    seq.dma_start(out=ft[32 * p:32 * p + 18, fsl],
                                  in_=feat_t.ap()[u][p][:, fsl])
                if j == 1:
                    # gemm2/gemm3 weights, needed a few chunks in
                    nc.gpsimd.dma_start(out=wt[:, 128:WCOLS],
                                        in_=wpack_t.ap()[u][:, 128:WCOLS])
            tg = tg_tiles[u % 2]
            for q in range(4):
                seq = nc.sync if q % 2 == 0 else nc.gpsimd
                seq.dma_start(out=tg[32 * q:32 * q + GRP, :],
                              in_=tgt_t.ap()[u][q])

            b1ap = bt[:, 0:1]
            b2ap = bt[:, 1:2]
            b3ap = bt[:, 2:3]

            h1 = h1pool.tile([128, HALF], F16, tag="h1")
            h2 = h2pool.tile([128, HALF], F16, tag="h2")

            def gemm1_chunk(j):
                # full-array matmuls ([18, 128] stationary, one MM per 512 px)
                # alternating the two lhsT1 band copies so LDWEIGHTS overlaps
                # the other band's in-flight matmul.
                pa = ps.tile([128, 1024], F32, tag="ps")
                for s in range(2):
                    t = 2 * j + s
                    p, c0 = t % 2, 512 * (t // 2)
                    nc.tensor.matmul(
                        pa[:, 512 * s:512 * s + 512],
                        wt[32 * p:32 * p + 18, 0:128],
                        ft[32 * p:32 * p + 18, c0:c0 + 512],
                        start=True, stop=True)
                dst = h1[:, 1024 * j:1024 * j + 1024]
                # ACT is ~15% faster per evac than DVE, so it gets 17 of the
                # 32 h-evacs per core.
                if (j + u) % 2 == 1 or (u, j) == (1, 1):
                    nc.scalar.activation(dst, pa, ACTF.Relu, bias=b1ap, scale=1.0)
                else:
                    nc.vector.tensor_scalar(out=dst, in0=pa, scalar1=b1ap,
                                            scalar2=0.0, op0=ALU.add, op1=ALU.max)

            def gemm2_chunk(j):
                # 4 concurrent diagonal 32x32 PE tiles per 512-px moving tile
                pb = ps.tile([128, 1024], F32, tag="ps")
                for s in range(2):
                    t = 2 * j + s
                    for r in range(4):
                        nc.tensor.matmul(
                            pb[32 * r:32 * r + 32, 512 * s:512 * s + 512],
                            wt[32 * r:32 * r + 32, 128:160],
                            h1[32 * r:32 * r + 32, 512 * t:512 * t + 512],
                            start=True, stop=True, tile_position=(32 * r, 32 * r))
                dst = h2[:, 1024 * j:1024 * j + 1024]
                if (j + u) % 2 == 0:
                    nc.scalar.activation(dst, pb, ACTF.Relu, bias=b2ap, scale=1.0)
                else:
                    nc.vector.tensor_scalar(out=dst, in0=pb, scalar1=b2ap,
                                            scalar2=0.0, op0=ALU.add, op1=ALU.max)

            def phase_c(kk):
                # gemm3 column-tiled (4 concurrent col groups), quadrant-
                # packed per half-unit into a [128, 1024] PSUM slot: tile
                # t = 4k+q lands at partition band 32q, col 512(k%2).  lhsT3
                # cols 160+8:192 are zero so filler rows are exact zeros and
                # the -50 bias drives them to sigmoid ~= 0.
                pc = ps.tile([128, 1024], F32, tag="ps")
                for k2 in range(2):
                    for q in range(4):
                        t = 4 * (2 * kk + k2) + q
                        nc.tensor.matmul(
                            pc[32 * q:32 * q + 32, 512 * k2:512 * k2 + 512],
                            wt[:, 160:192], h2[:, 512 * t:512 * t + 512],
                            start=True, stop=True, tile_position=(0, 32 * q))
                pred = ppool.tile([128, 1024], F16, tag="p")
                nc.scalar.activation(pred, pc, ACTF.Sigmoid, bias=b3ap, scale=1.0)
                tgs = tg[:, 1024 * kk:1024 * kk + 1024]
                col = 2 * u + kk
                prod = spool.tile([128, 1024], F16, tag="s")
                nc.vector.scalar_tensor_tensor(
                    out=prod, in0=pred, scalar=0.0, in1=tgs,
                    op0=ALU.add, op1=ALU.mult,
                    accum_out=acc[:, col:col + 1])
                sq = spool.tile([128, 1024], F16, tag="s")
                pcol = 2 * NG + col
                if (u + kk) % 2 == 0:
                    nc.scalar.activation(sq, pred, ACTF.Square,
                                         accum_out=acc[:, pcol:pcol + 1])
                else:
                    nc.vector.scalar_tensor_tensor(
                        out=sq, in0=pred, scalar=0.0, in1=pred,
                        op0=ALU.add, op1=ALU.mult,
                        accum_out=acc[:, pcol:pcol + 1])

            # interleaved emission at 2-chunk granularity: keeps gemm2's
            # diagonal tile groups adjacent (they overlap across groups on
            # the PE) while the in-order PE still alternates phases so both
            # evac engines get work from the start.
            for s2 in range(5):
                if s2 < 4:
                    gemm1_chunk(2 * s2)
                    gemm1_chunk(2 * s2 + 1)
                if s2 == 3:
                    phase_c(0)
                if s2 >= 1:
                    gemm2_chunk(2 * (s2 - 1))
                    gemm2_chunk(2 * (s2 - 1) + 1)
            phase_c(1)

        nc.sync.dma_start(out=acc_t.ap(), in_=acc)

    nc.compile()
    _PROGRAM_CACHE[NG] = nc
    return nc


def _run(inputs, trace=False):
    seg_feat = np.asarray(inputs["seg_feat"], np.float32)
    conv_weight = np.asarray(inputs["conv_weight"], np.float32)
    mask = np.asarray(inputs["mask"])
    ind = np.asarray(inputs["ind"])
    target = np.asarray(inputs["target"], np.float32)

    in_maps, per_core, NG = host_pack(seg_feat, conv_weight, mask, ind, target)
    nc = build_program(NG)
    res = run_bass_kernel_spmd(nc, in_maps, core_ids=list(range(N_CORES)),
                               trace=trace)

    inter = np.zeros(B, np.float64)
    predsq = np.zeros(B, np.float64)
    for ci in range(N_CORES):
        acc = res.results[ci]["acc"]
        NGc = acc.shape[1] // 4
        for u, (b, grp, half) in enumerate(per_core[ci]):
            if all(o < 0 for o in grp):
                continue
            inter[b] += acc[:, 2 * u:2 * u + 2].sum(dtype=np.float64)
            predsq[b] += acc[:, 2 * NGc + 2 * u:2 * NGc + 2 * u + 2].sum(
                dtype=np.float64)
    tgtsq = ((target.reshape(B, O, HW).astype(np.float64) ** 2)
             * mask[:, :, None]).sum(axis=(1, 2))
    loss = 1.0 - (2.0 * inter + 1.0) / (predsq + tgtsq + 1.0)
    return np.float32(loss.mean()), res


def kernel(**inputs):
    loss, _ = _run(inputs, trace=False)
    return np.array(loss, dtype=np.float32)


# revision 26
# speedup vs baseline: 1.1442x; 1.1442x over previous
# Fused dynamic-conv (CondInst-style) + dice loss kernel for 8x TRN2 NeuronCores.
#
# Reference computation (per batch image b, object o):
#   weight[b,o,:] = conv_weight[b, :, ind[b,o]]           (gather, 593 params)
#   feat = concat(seg_feat[b], x_rel(o), y_rel(o))        ([18, 128*128])
#   h1 = relu(w1 @ feat + b1); h2 = relu(w2 @ h1 + b2)    (16-ch dynamic 1x1 convs)
#   out = sigmoid(w3 . h2 + b3)                           ([128*128])
#   dice over masked objects -> scalar loss
#
# Strategy (v3 — evacuation-balanced pipeline):
#  * Host gathers the 593 dynamic params per object and packs active (mask=1)
#    objects into groups of 8; x_rel/y_rel fold into an effective bias b1_eff.
#    One shared [18, HW] feature map per image serves all its objects.
#  * Work unit = (group, half-image) = 8192 px; 16 units over 8 cores (NG=2).
#  * Matmuls run as 4 concurrent diagonal/column 32-wide PE tiles (full column
#    rate).  PSUM = 4 rotating [128, 1024] slots (2 banks each): fill is 8 MMs
#    (~0.5us), drain is ONE FD=1024 evacuation op (~1.1us), so two slots are
#    always draining on the two evac engines concurrently.
#  * The evacuations (relu+bias) are the roofline: ~45K FD-cycles/core split
#    across ACT and DVE; assignments alternate strictly between the engines
#    (parity flipped per unit/phase) to keep both ~balanced (~25us each).
#  * gemm3 is column-tiled and quadrant-packed per half-unit into [128, 1024]
#    PSUM slots; sigmoid evacuates fp16 pred; dice partials via accum_out:
#    sum pred*tgt (DVE scalar_tensor_tensor) and sum pred^2 (Square on ACT /
#    STT on DVE, alternating per unit).  Host does the final tiny reduction
#    plus sum(tgt^2), which is network-independent.
import numpy as np
from contextlib import ExitStack

import concourse.bass as bass
import concourse.tile as tile
from concourse import mybir, bacc
from concourse.bass_utils import run_bass_kernel_spmd

C = 16
WT = 593
B, O, H, W = 4, 32, 128, 128
HW = H * W
N_CORES = 8
GRP = 8            # objects per block-diagonal group
HALF = 8192        # pixels per work unit

F32 = mybir.dt.float32
F16 = mybir.dt.float16
ACTF = mybir.ActivationFunctionType
ALU = mybir.AluOpType

# wpack free-dim layout (per unit, [128, 192] float16):
#   0:128   lhsT1 [18, 128]: full-array gemm1, col 16o+c = w1[o][c, :].
#           Duplicated at partition rows 32:50 — gemm1 matmuls alternate the
#           two copies (different PE row groups) so each LDWEIGHTS overlaps
#           the other copy's in-flight matmul instead of serializing.
#   128:160 lhsT2 band blocks: rows 32r hold diag(w2[2r]^T, w2[2r+1]^T)
#           (gemm2 runs as 4 concurrent diagonal 32x32 PE tiles)
#   160:192 lhsT3 (block-diag w3; cols 160+8:192 zero)
# bias layout (per unit, [128, 3] float32): 0 = b1_eff, 1 = b2, 2 = b3/-50
# feat layout: [NG, 2, 18, 4096] — band p holds the 512-px moving tiles with
# t % 2 == p (concatenated), matching the gemm1 band alternation.
WCOLS = 192


def host_pack(seg_feat, conv_weight, mask, ind, target):
    cw = conv_weight.reshape(B, WT, HW)
    weight = np.take_along_axis(cw, ind[:, None, :].astype(np.int64), axis=2)
    weight = np.ascontiguousarray(weight.transpose(0, 2, 1))  # [B, O, WT]
    s0 = (C + 2) * C
    w1 = weight[..., :s0].reshape(B, O, C, C + 2)
    b1 = weight[..., s0:s0 + C]
    w2 = weight[..., s0 + C:s0 + C + C * C].reshape(B, O, C, C)
    b2 = weight[..., s0 + C + C * C:s0 + 2 * C + C * C]
    w3 = weight[..., s0 + 2 * C + C * C:s0 + 3 * C + C * C]
    b3 = weight[..., -1]
    xo = (ind % W).astype(np.float32)
    yo = (ind // W).astype(np.float32)

    units = []  # (b, objs[8 padded with -1], half)
    for b in range(B):
        objs = [o for o in range(O) if mask[b, o] == 1]
        for g0 in range(0, len(objs), GRP):
            grp = objs[g0:g0 + GRP]
            grp = grp + [-1] * (GRP - len(grp))
            for half in range(2):
                units.append((b, grp, half))
    per_core = [[] for _ in range(N_CORES)]
    for i, u in enumerate(units):
        per_core[i % N_CORES].append(u)
    NG = max(1, max(len(u) for u in per_core))
    for ci in range(N_CORES):
        while len(per_core[ci]) < NG:
            per_core[ci].append((0, [-1] * GRP, 0))

    px = np.arange(HW, dtype=np.float32)
    xg = (px % W) / 128.0
    yg = np.floor(px / W) / 128.0
    tgt_flat = target.reshape(B, O, HW)

    in_maps = []
    for ci in range(N_CORES):
        feat_pack = np.zeros((NG, 2, 18, HALF // 2), np.float16)
        wpack = np.zeros((NG, 128, WCOLS), np.float16)
        bias_pack = np.zeros((NG, 128, 3), np.float32)
        # tgt dense in the packed sigmoid layout: row (q, o) maps to
        # partition 32q+o; pred[32q+o, 512k+j] holds pixel 2048k + 512q + j
        # of object o  (t = 4k+q moving tiles)
        tgt_pack = np.zeros((NG, 4, GRP, 2048), np.float16)
        for u, (b, grp, half) in enumerate(per_core[ci]):
            sl = slice(half * HALF, (half + 1) * HALF)
            fu = np.empty((18, HALF), np.float16)
            fu[:16] = seg_feat[b].reshape(C, HW)[:, sl]
            fu[16] = xg[sl]
            fu[17] = yg[sl]
            # band p = 512-px tiles t with t % 2 == p, concatenated
            ft4 = fu.reshape(18, 16, 512)
            feat_pack[u, 0] = ft4[:, 0::2].reshape(18, HALF // 2)
            feat_pack[u, 1] = ft4[:, 1::2].reshape(18, HALF // 2)
            bias_pack[u, :, 2] = -50.0  # filler-row sigmoid bias
            for oo, o in enumerate(grp):
                if o < 0:
                    continue
                r, p = oo // 2, oo % 2
                w1T = w1[b, o].T.astype(np.float16)
                wpack[u, 0:18, 16 * oo:16 * oo + 16] = w1T
                wpack[u, 32:50, 16 * oo:16 * oo + 16] = w1T
                b1e = (b1[b, o] - w1[b, o, :, 16] * (xo[b, o] / 128.0)
                       - w1[b, o, :, 17] * (yo[b, o] / 128.0))
                bias_pack[u, 16 * oo:16 * oo + 16, 0] = b1e
                wpack[u, 32 * r + 16 * p:32 * r + 16 * p + 16,
                      128 + 16 * p:128 + 16 * p + 16] = \
                    w2[b, o].T.astype(np.float16)
                bias_pack[u, 16 * oo:16 * oo + 16, 1] = b2[b, o]
                wpack[u, 16 * oo:16 * oo + 16, 160 + oo] = \
                    w3[b, o].astype(np.float16)
                for q in range(4):
                    bias_pack[u, 32 * q + oo, 2] = b3[b, o]
                for t in range(16):
                    q, k = t % 4, t // 4
                    g0 = half * HALF + t * 512
                    tgt_pack[u, q, oo, 512 * k:512 * k + 512] = \
                        tgt_flat[b, o, g0:g0 + 512].astype(np.float16)
        in_maps.append({"feat": feat_pack, "wpack": wpack,
                        "bias": bias_pack, "tgt": tgt_pack})
    return in_maps, per_core, NG


_PROGRAM_CACHE = {}


def build_program(NG):
    if NG in _PROGRAM_CACHE:
        return _PROGRAM_CACHE[NG]
    nc = bacc.Bacc("TRN2", target_bir_lowering=False, debug=False,
                   enable_asserts=False, num_devices=N_CORES)
    feat_t = nc.dram_tensor("feat", (NG, 2, 18, HALF // 2), F16,
                            kind="ExternalInput")
    wpack_t = nc.dram_tensor("wpack", (NG, 128, WCOLS), F16, kind="ExternalInput")
    bias_t = nc.dram_tensor("bias", (NG, 128, 3), F32, kind="ExternalInput")
    tgt_t = nc.dram_tensor("tgt", (NG, 4, GRP, 2048), F16,
                           kind="ExternalInput")
    acc_t = nc.dram_tensor("acc", (128, 4 * NG), F32, kind="ExternalOutput")

    with tile.TileContext(nc) as tc, ExitStack() as ctx:
        wpool = ctx.enter_context(tc.tile_pool(name="wpool", bufs=2))
        fpool = ctx.enter_context(tc.tile_pool(name="fpool", bufs=2))
        h1pool = ctx.enter_context(tc.tile_pool(name="h1pool", bufs=2))
        h2pool = ctx.enter_context(tc.tile_pool(name="h2pool", bufs=2))
        tpool = ctx.enter_context(tc.tile_pool(name="tpool", bufs=2))
        ppool = ctx.enter_context(tc.tile_pool(name="ppool", bufs=2))
        spool = ctx.enter_context(tc.tile_pool(name="spool", bufs=4))
        apool = ctx.enter_context(tc.tile_pool(name="apool", bufs=1))
        ps = ctx.enter_context(tc.tile_pool(name="ps", bufs=4, space="PSUM"))

        # one accumulator tile: cols [0, 2NG) = inter, [2NG, 4NG) = predsq
        acc = apool.tile([128, 4 * NG], F32)

        tg_tiles = [tpool.tile([128, 2048], F16, tag="t", name=f"tg{i}")
                    for i in range(2)]

        # Warm-up during the initial DMA wait: load the sigmoid table set
        # (covers relu/sigmoid/square, so no mid-kernel table switch) and run
        # dummy matmuls back-to-back so the PE HAM clock-gate opens (needs
        # ~3.4us of sustained PE activity) before the real matmuls arrive.
        scr = apool.tile([128, 512], F16)
        nc.vector.memset(scr, 0.125)
        scr1 = apool.tile([128, 1], F32)
        nc.scalar.activation(scr1, scr[:, 0:1], ACTF.Sigmoid, bias=0.0, scale=1.0)
        pw = ps.tile([128, 1024], F32, tag="ps")
        for i in range(8):
            nc.tensor.matmul(pw[:, 512 * (i % 2):512 * (i % 2) + 512],
                             scr[:, 0:128], scr, start=True, stop=True)

        for u in range(NG):
            wt = wpool.tile([128, WCOLS], F16, tag="w")
            bt = wpool.tile([128, 3], F32, tag="b")
            nc.gpsimd.dma_start(out=bt, in_=bias_t.ap()[u])
            # gemm1 weights are only rows 0:50 of cols 0:128 — ship just
            # those (12.8KB) so the first matmul isn't gated on a big DMA.
            # (NOT on the scalar queue: ACT-queue DMAs force a spurious ACT
            # table load.)
            nc.sync.dma_start(out=wt[0:50, 0:128],
                              in_=wpack_t.ap()[u][0:50, 0:128])
            ft = fpool.tile([50, HALF // 2], F16, tag="f")
            # fine-grained feat DMAs (per band per chunk), first pixel-chunk
            # first; the very first chunk is split in half again.  Issues
            # alternate sync/gpsimd to parallelize issue + transfer.
            for p in range(2):
                seq = nc.sync if p == 0 else nc.gpsimd
                seq.dma_start(out=ft[32 * p:32 * p + 18, 0:512],
                              in_=feat_t.ap()[u][p][:, 0:512])
                seq.dma_start(out=ft[32 * p:32 * p + 18, 512:1024],
                              in_=feat_t.ap()[u][p][:, 512:1024])
            for j in range(1, 4):
                fsl = slice(1024 * j, 1024 * j + 1024)
                for p in range(2):
                    seq = nc.sync if p == 0 else nc.gpsimd
                    seq.dma_start(out=ft[32 * p:32 * p + 18, fsl],
                                  in_=feat_t.ap()[u][p][:, fsl])
                if j == 1:
                    # gemm2/gemm3 weights, needed a few chunks in
                    nc.gpsimd.dma_start(out=wt[:, 128:WCOLS],
                                        in_=wpack_t.ap()[u][:, 128:WCOLS])
            tg = tg_tiles[u % 2]
            if u < 2:
                # zero once per slot: the dense tgt DMAs only overwrite the 8
                # real rows per quadrant; filler rows must read as 0.0
                nc.gpsimd.memset(tg, 0.0)
            for q in range(4):
                nc.gpsimd.dma_start(out=tg[32 * q:32 * q + GRP, :],
                                    in_=tgt_t.ap()[u][q])

            b1ap = bt[:, 0:1]
            b2ap = bt[:, 1:2]
            b3ap = bt[:, 2:3]

            h1 = h1pool.tile([128, HALF], F16, tag="h1")
            h2 = h2pool.tile([128, HALF], F16, tag="h2")

            def gemm1_chunk(j):
                # full-array matmuls ([18, 128] stationary, one MM per 512 px)
                # alternating the two lhsT1 band copies so LDWEIGHTS overlaps
                # the other band's in-flight matmul.
                pa = ps.tile([128, 1024], F32, tag="ps")
                for s in range(2):
                    t = 2 * j + s
                    p, c0 = t % 2, 512 * (t // 2)
                    nc.tensor.matmul(
                        pa[:, 512 * s:512 * s + 512],
                        wt[32 * p:32 * p + 18, 0:128],
                        ft[32 * p:32 * p + 18, c0:c0 + 512],
                        start=True, stop=True)
                dst = h1[:, 1024 * j:1024 * j + 1024]
                # ACT is ~15% faster per evac than DVE, so it gets 17 of the
                # 32 h-evacs per core.
                if (j + u) % 2 == 1 or (u, j) == (1, 1):
                    nc.scalar.activation(dst, pa, ACTF.Relu, bias=b1ap, scale=1.0)
                else:
                    nc.vector.tensor_scalar(out=dst, in0=pa, scalar1=b1ap,
                                            scalar2=0.0, op0=ALU.add, op1=ALU.max)

            def gemm2_chunk(j):
                # 4 concurrent diagonal 32x32 PE tiles per 512-px moving tile
                pb = ps.tile([128, 1024], F32, tag="ps")
                for s in range(2):
                    t = 2 * j + s
                    for r in range(4):
                        nc.tensor.matmul(
                            pb[32 * r:32 * r + 32, 512 * s:512 * s + 512],
                            wt[32 * r:32 * r + 32, 128:160],
                            h1[32 * r:32 * r + 32, 512 * t:512 * t + 512],
                            start=True, stop=True, tile_position=(32 * r, 32 * r))
                dst = h2[:, 1024 * j:1024 * j + 1024]
                if (j + u) % 2 == 0:
                    nc.scalar.activation(dst, pb, ACTF.Relu, bias=b2ap, scale=1.0)
                else:
                    nc.vector.tensor_scalar(out=dst, in0=pb, scalar1=b2ap,
                                            scalar2=0.0, op0=ALU.add, op1=ALU.max)

            def phase_c(kk):
                # gemm3 column-tiled (4 concurrent col groups), quadrant-
                # packed per half-unit into a [128, 1024] PSUM slot: tile
                # t = 4k+q lands at partition band 32q, col 512(k%2).  lhsT3
                # cols 160+8:192 are zero so filler rows are exact zeros and
                # the -50 bias drives them to sigmoid ~= 0.
                pc = ps.tile([128, 1024], F32, tag="ps")
                for k2 in range(2):
                    for q in range(4):
                        t = 4 * (2 * kk + k2) + q
                        nc.tensor.matmul(
                            pc[32 * q:32 * q + 32, 512 * k2:512 * k2 + 512],
                            wt[:, 160:192], h2[:, 512 * t:512 * t + 512],
                            start=True, stop=True, tile_position=(0, 32 * q))
                pred = ppool.tile([128, 1024], F16, tag="p")
                nc.scalar.activation(pred, pc, ACTF.Sigmoid, bias=b3ap, scale=1.0)
                tgs = tg[:, 1024 * kk:1024 * kk + 1024]
                col = 2 * u + kk
                prod = spool.tile([128, 1024], F16, tag="s")
                nc.vector.scalar_tensor_tensor(
                    out=prod, in0=pred, scalar=0.0, in1=tgs,
                    op0=ALU.add, op1=ALU.mult,
                    accum_out=acc[:, col:col + 1])
                sq = spool.tile([128, 1024], F16, tag="s")
                pcol = 2 * NG + col
                if (u + kk) % 2 == 0:
                    nc.scalar.activation(sq, pred, ACTF.Square,
                                         accum_out=acc[:, pcol:pcol + 1])
                else:
                    nc.vector.scalar_tensor_tensor(
                        out=sq, in0=pred, scalar=0.0, in1=pred,
                        op0=ALU.add, op1=ALU.mult,
                        accum_out=acc[:, pcol:pcol + 1])

            # interleaved emission at 2-chunk granularity: keeps gemm2's
            # diagonal tile groups adjacent (they overlap across groups on
            # the PE) while the in-order PE still alternates phases so both
            # evac engines get work from the start.
            for s2 in range(5):
                if s2 < 4:
                    gemm1_chunk(2 * s2)
                    gemm1_chunk(2 * s2 + 1)
                if s2 == 3:
                    phase_c(0)
                if s2 >= 1:
                    gemm2_chunk(2 * (s2 - 1))
                    gemm2_chunk(2 * (s2 - 1) + 1)
            phase_c(1)

        nc.sync.dma_start(out=acc_t.ap(), in_=acc)

    nc.compile()
    _PROGRAM_CACHE[NG] = nc
    return nc


def _run(inputs, trace=False):
    seg_feat = np.asarray(inputs["seg_feat"], np.float32)
    conv_weight = np.asarray(inputs["conv_weight"], np.float32)
    mask = np.asarray(inputs["mask"])
    ind = np.asarray(inputs["ind"])
    target = np.asarray(inputs["target"], np.float32)

    in_maps, per_core, NG = host_pack(seg_feat, conv_weight, mask, ind, target)
    nc = build_program(NG)
    res = run_bass_kernel_spmd(nc, in_maps, core_ids=list(range(N_CORES)),
                               trace=trace)

    inter = np.zeros(B, np.float64)
    predsq = np.zeros(B, np.float64)
    for ci in range(N_CORES):
        acc = res.results[ci]["acc"]
        NGc = acc.shape[1] // 4
        for u, (b, grp, half) in enumerate(per_core[ci]):
            if all(o < 0 for o in grp):
                continue
            inter[b] += acc[:, 2 * u:2 * u + 2].sum(dtype=np.float64)
            predsq[b] += acc[:, 2 * NGc + 2 * u:2 * NGc + 2 * u + 2].sum(
                dtype=np.float64)
    tgtsq = ((target.reshape(B, O, HW).astype(np.float64) ** 2)
             * mask[:, :, None]).sum(axis=(1, 2))
    loss = 1.0 - (2.0 * inter + 1.0) / (predsq + tgtsq + 1.0)
    return np.float32(loss.mean()), res


def kernel(**inputs):
    loss, _ = _run(inputs, trace=False)
    return np.array(loss, dtype=np.float32)
